# revision 33
# baseline (speedup 1.0000x reference)
"""DualTierMiras Trainium2 kernel (8-core data-parallel), v10 (~184us).

Math (per row r of B=65536, D=256, H=4 heads, hd=64, S=64 keys, 2 banks):
  q = query @ Wq.T
  per head h, bank t: sim = (q_h/|q_h|) . kn_t[h,s,:]   (kn = normalized keys)
  attn = softmax_s(sim);  v_t[h] = attn @ vals_t[h]
  mix  = sigmoid(mix_logit + mean(tanh(context @ Wg.T)))
  conf = sigmoid(Wc2 @ tanh(Wc1 @ context) + bc2)
  out  = (conf*mix*v_fast + conf*(1-mix)*v_deep) @ Wo.T

v3 structure (per core 8192 rows; macro = 512 rows, 4 subtiles):
  - query path (sims + q-norm projections) runs in fp8e4m3 DoubleRow
    matmuls (K=256 folded into one PE pass). wqt/mfd are scaled x16 on the
    host so fp8 stays in normal range; the x16 cancels exactly through the
    1/|q| softmax temperature.
  - context path (gate/conf) and the AV matmuls stay f16 for accuracy.
  - PE stream for step m interleaves transposes/AV of macro m-2 into the
    sims/gc/qp stream of macro m (2-step lag) -> no head-of-line blocking.
  - q-norms: qp/squares run 4 steps ahead; the norm reduce is deferred
    one further step so it fills the vector bubble while scalar runs exp;
    sqrt+recip batched per 3 macros (ACT table set cycles only 8x --
    Sqrt lives in a different table set than Tanh/Exp/Square).
  - all row-stat reduces write f16 outs; output is f16 (host upcasts).
  - e-tile triple-buffered (exp must not wait on the gpsimd ep multiply of
    macro m-2); conf reduction emitted late in the vector FIFO; epilogue
    interleaves the last two macros; critical-path DMAs (wqt8, x8 chunk 0)
    issued first.
"""

import sys

import numpy as np

sys.path.insert(0, "/opt/trn_rl_repo")

from contextlib import ExitStack

import ml_dtypes

import concourse.mybir as mybir
from concourse import bacc, tile
from concourse.bass_utils import run_bass_kernel_spmd

F32 = mybir.dt.float32
F16 = mybir.dt.float16
F8 = mybir.dt.float8e4

N_CORES = 8
B, D, H, S, HD = 65536, 256, 4, 64, 64
RPC = B // N_CORES            # rows per core
MACRO = 512                   # rows per macro tile
SUB = 128                     # rows per sub tile
N_MACRO = RPC // MACRO
N_SUB = MACRO // SUB
EPS = 1e-8
W8SCALE = 16.0                # fp8 weight pre-scale (cancels via 1/|q|)

MM_DT = F16

AF = mybir.ActivationFunctionType
ALU = mybir.AluOpType
DR = mybir.MatmulPerfMode.DoubleRow

NP_F8 = ml_dtypes.float8_e4m3


def to_mm(x):
    return np.ascontiguousarray(x, np.float16)


def _build_kernel(tc, ctx, io, n_macro=N_MACRO):
    nc = tc.nc
    (x8_d, cT_d, wqt8_d, mfd8_d, wgc_d, v4_d, wc2r_d, misc_d,
     nra_d, nrb_d, ident_d, out_d) = io

    consts = ctx.enter_context(tc.tile_pool(name="consts", bufs=1))
    wqt8 = consts.tile([128, 2, 256], F8, tag="wqt8", name="wqt8")
    mfd8 = consts.tile([128, 2, 512], F8, tag="mfd8", name="mfd8")
    wgc = [consts.tile([128, 384], MM_DT, tag=f"wgc{k}", name=f"wgc{k}") for k in range(2)]
    v4 = [consts.tile([128, 256], MM_DT, tag=f"v4{q}", name=f"v4{q}") for q in range(4)]
    wc2r = consts.tile([128, 128], F16, tag="wc2r", name="wc2r")
    misc = consts.tile([128, 4], F32, tag="misc", name="misc")  # col0 mix/2, col1 bc2/2
    nra = consts.tile([128, 4], F32, tag="nra", name="nra")  # 1.5/sqrt(E_h)
    nrb = consts.tile([128, 4], F32, tag="nrb", name="nrb")  # 0.5/E_h^1.5
    ident = consts.tile([128, 128], MM_DT, tag="ident", name="ident")

    rows = n_macro * MACRO

    # resident x (fp8, DoubleRow layout [128, 2, rows]) in per-1024 chunks
    XCH = 1024
    n_xch = rows // XCH
    xin = ctx.enter_context(tc.tile_pool(name="xin", bufs=1))
    x8c = [xin.tile([128, 2, XCH], F8, tag=f"x8_{c}", name=f"x8_{c}")
           for c in range(n_xch)]
    # critical-path DMAs first: qp(0) needs wqt8 + x8 chunk 0
    nc.sync.dma_start(wqt8[:], wqt8_d[:])
    for c in range(2):
        nc.sync.dma_start(x8c[c][:], x8_d[:, :, c * XCH:(c + 1) * XCH])
    nc.sync.dma_start(mfd8[:], mfd8_d[:])
    for k in range(2):
        nc.sync.dma_start(wgc[k][:], wgc_d[k])
    for q in range(4):
        nc.sync.dma_start(v4[q][:], v4_d[q])
    nc.sync.dma_start(wc2r[:], wc2r_d[:])
    nc.sync.dma_start(misc[:], misc_d[:])
    nc.sync.dma_start(nra[:], nra_d[:])
    nc.sync.dma_start(nrb[:], nrb_d[:])
    nc.sync.dma_start(ident[:], ident_d[:])
    for c in range(2, n_xch):
        nc.sync.dma_start(x8c[c][:], x8_d[:, :, c * XCH:(c + 1) * XCH])

    def x8slice(m, lo, hi):
        c = (m * MACRO) // XCH
        o = (m * MACRO) % XCH
        return x8c[c][:, :, o + lo:o + hi]

    cin = ctx.enter_context(tc.tile_pool(name="cin", bufs=4))
    # psum pools: 7 of 8 banks
    ps_a = ctx.enter_context(tc.tile_pool(name="ps_a", bufs=2, space="PSUM"))
    ps_gc = ctx.enter_context(tc.tile_pool(name="ps_gc", bufs=1, space="PSUM"))
    ps_et = ctx.enter_context(tc.tile_pool(name="ps_et", bufs=1, space="PSUM"))
    ps_fin = ctx.enter_context(tc.tile_pool(name="ps_fin", bufs=2, space="PSUM"))

    big = ctx.enter_context(tc.tile_pool(name="big", bufs=2))
    sml = ctx.enter_context(tc.tile_pool(name="sml", bufs=3))
    outp = ctx.enter_context(tc.tile_pool(name="outp", bufs=4))
    stat = ctx.enter_context(tc.tile_pool(name="stat", bufs=1))

    # resident norm stats for the whole core
    ssq_all = stat.tile([128, 16 * n_macro], F32, tag="ssq_all", name="ssq_all")
    invna_all = stat.tile([128, 16 * n_macro], F32, tag="invna_all",
                          name="invna_all")

    st = {}

    def sims_mm(m, i):
        s = st.setdefault(m, {})
        if "sim" not in s:
            s["sim"] = [None] * N_SUB
        sim = ps_a.tile([128, 512], F32, tag="blk", name="sim")
        nc.tensor.matmul(sim[:], x8slice(m, i * SUB, (i + 1) * SUB), mfd8[:],
                         start=True, stop=True, perf_mode=DR)
        s["sim"][i] = sim

    def gc_mm(m, p):
        s = st[m]
        if "gc" not in s:
            s["gc"] = [None, None]
        ct = s["ct"]
        gcp = ps_gc.tile([128, 1024], F32, tag="gcp", name="gcp")
        for j in range(2):
            i = 2 * p + j
            sl = slice(i * SUB, (i + 1) * SUB)
            o = j * 512
            nc.tensor.matmul(gcp[:, o:o + 384], ct[0][:, sl], wgc[0][:],
                             start=True, stop=False)
            nc.tensor.matmul(gcp[:, o:o + 384], ct[1][:, sl], wgc[1][:],
                             start=False, stop=True)
        s["gc"][p] = gcp

    def qp_mm(m):
        """fp8-DR q projections for macro m (emitted 4 steps ahead)."""
        s = st.setdefault(m, {})
        s["qp"] = []
        for p in range(2):
            qpr = ps_a.tile([128, 512], F32, tag="blk", name="qpr")
            for j in range(2):
                i = 2 * p + j
                nc.tensor.matmul(qpr[:, j * 256:(j + 1) * 256],
                                 x8slice(m, i * SUB, (i + 1) * SUB), wqt8[:],
                                 start=True, stop=True, perf_mode=DR)
            s["qp"].append(qpr)

    def norms_sq(m):
        """squares (scalar) for macro m's q projections."""
        s = st[m]
        qsq = big.tile([128, 1024], F16, tag="qsq", name="qsq")
        for p in range(2):
            nc.scalar.activation(qsq[:, p * 512:(p + 1) * 512], s["qp"][p][:],
                                 AF.Square)
        s["qsq"] = qsq

    def norms_red(m):
        """per-head reduce into ssq_all (deferred one step: fills the
        vector bubble while scalar runs the current macro's exp)."""
        nc.vector.reduce_sum(
            ssq_all[:, m * 16:(m + 1) * 16],
            st[m]["qsq"][:].rearrange("p (g s) -> p g s", g=16),
            axis=mybir.AxisListType.X)

    def norms_rsqrt_gp(m0, m1):
        """invna = rsqrt(ssq) via gpsimd Newton (4 eff. iterations from a
        per-head constant seed; iter 1 folded into nra/nrb). Avoids the
        in-loop ACT Sqrt table switches and the vector reciprocal."""
        k = (m1 - m0) * 16
        sl = slice(m0 * 16, m1 * 16)
        x = ssq_all[:, sl].rearrange("p (g h) -> p g h", h=4)
        y = sml.tile([128, 48], F32, tag="nry", name="nry",
                     padded_shape=[128, 48])
        s = sml.tile([128, 48], F32, tag="nrs", name="nrs",
                     padded_shape=[128, 48])
        g = k // 4
        yv = y[:, 0:k].rearrange("p (g h) -> p g h", h=4)
        sv = s[:, 0:k].rearrange("p (g h) -> p g h", h=4)
        # y1 = a - b*x  (a, b broadcast per head)
        nc.gpsimd.tensor_tensor(
            sv, x, nrb[:].unsqueeze(1).broadcast_to([128, g, 4]), ALU.mult)
        nc.gpsimd.tensor_tensor(
            yv, nra[:].unsqueeze(1).broadcast_to([128, g, 4]), sv,
            ALU.subtract)
        for it in range(3):
            last = it == 2
            out = invna_all[:, sl].rearrange("p (g h) -> p g h", h=4)                 if last else yv
            nc.gpsimd.tensor_tensor(sv, yv, yv, ALU.mult)
            nc.gpsimd.tensor_tensor(sv, sv, x, ALU.mult)
            nc.gpsimd.tensor_scalar(sv, sv, -0.5, 1.5, ALU.mult, ALU.add)
            nc.gpsimd.tensor_tensor(out, yv, sv, ALU.mult)

    def norms_rsqrt(m0, m1):
        """batched sqrt+recip for macros [m0, m1) -> invna_all."""
        sl = slice(m0 * 16, m1 * 16)
        sna = sml.tile([128, 16 * 4], F32, tag="sna", name="sna",
                       padded_shape=[128, 64])
        w = (m1 - m0) * 16
        nc.scalar.activation(sna[:, 0:w], ssq_all[:, sl], AF.Sqrt)
        nc.vector.reciprocal(invna_all[:, sl], sna[:, 0:w])

    def v1_mm(m, i):
        """vector: earg slice = sim * invn (per subtile)."""
        s = st[m]
        if "earg" not in s:
            s["earg"] = big.tile([128, 2048], F16, tag="earg", name="earg")
        nc.vector.tensor_tensor(
            s["earg"][:, i * 512:(i + 1) * 512]
            .rearrange("p (t h s) -> p t h s", t=2, h=4),
            s["sim"][i][:].rearrange("p (t h s) -> p t h s", t=2, h=4),
            invna_all[:, m * 16 + i * 4:m * 16 + (i + 1) * 4]
            .unsqueeze(1).unsqueeze(3).broadcast_to([128, 2, 4, 64]),
            ALU.mult)

    def tanh_p(m, p):
        s = st[m]
        if "tg" not in s:
            s["tg"] = big.tile([128, 1536], F16, tag="tg", name="tg")
        nc.scalar.activation(
            s["tg"][:, p * 768:(p + 1) * 768].rearrange("p (j f) -> p j f", j=2),
            s["gc"][p][:].rearrange("p (j f) -> p j f", j=2)[:, :, 0:384],
            AF.Tanh)

    def exp_p(m, p):
        s = st[m]
        if "e" not in s:
            s["e"] = big.tile([128, 2048], F16, tag="e", name="e", bufs=3)
        nc.scalar.activation(s["e"][:, p * 1024:(p + 1) * 1024],
                             s["earg"][:, p * 1024:(p + 1) * 1024], AF.Exp)

    def stats(m):
        """vector den/gate/conf reductions (+gpsimd conf product)."""
        s = st[m]
        e, tg = s["e"], s["tg"]
        den = sml.tile([128, 32], F16, tag="den", name="den")
        thin = sml.tile([128, 8], F16, tag="thin", name="thin")
        with nc.allow_low_precision(reason="f16 softmax stats (<=64 adds)"):
            nc.vector.reduce_sum(
                den[:], e[:].rearrange("p (g s) -> p g s", g=32),
                axis=mybir.AxisListType.X)
            nc.vector.reduce_sum(
                thin[:].rearrange("p (i two) -> p i two", two=2)[:, :, 0:1],
                tg[:].rearrange("p (i f) -> p i f", i=4)[:, :, 0:256]
                .rearrange("p i (one f) -> p i one f", one=1),
                axis=mybir.AxisListType.X)
        cp = big.tile([128, 512], F16, tag="cp", name="cp")
        nc.gpsimd.tensor_tensor(
            cp[:].rearrange("p (i f) -> p i f", i=4),
            tg[:].rearrange("p (i f) -> p i f", i=4)[:, :, 256:384],
            wc2r[:].unsqueeze(1).broadcast_to([128, 4, 128]),
            ALU.mult)
        s["cp"] = cp
        s["den"] = den
        s["thin"] = thin

    def conf_red(m):
        """conf reduction, emitted late so a slow gpsimd cp can't
        head-of-line-block the next step's v1s on the vector queue."""
        s = st[m]
        with nc.allow_low_precision(reason="f16 conf stat"):
            nc.vector.reduce_sum(
                s["thin"][:].rearrange("p (i two) -> p i two", two=2)[:, :, 1:2],
                s["cp"][:].rearrange("p (i one f) -> p i one f", i=4, one=1),
                axis=mybir.AxisListType.X)

    def soft_b(m):
        s = st[m]
        thin, den, e = s["thin"], s["den"], s["e"]
        th = sml.tile([128, 8], F32, tag="th", name="th")
        nc.scalar.activation(
            th[:].rearrange("p (i two) -> p i two", two=2)[:, :, 0:1],
            thin[:].rearrange("p (i two) -> p i two", two=2)[:, :, 0:1],
            AF.Tanh, bias=misc[:, 0:1], scale=1.0 / 512.0)
        nc.scalar.activation(
            th[:].rearrange("p (i two) -> p i two", two=2)[:, :, 1:2],
            thin[:].rearrange("p (i two) -> p i two", two=2)[:, :, 1:2],
            AF.Tanh, bias=misc[:, 1:2], scale=0.5)
        u = sml.tile([128, 4], F32, tag="u", name="u")
        nc.gpsimd.tensor_scalar(
            u[:], th[:].rearrange("p (i two) -> p i two", two=2)[:, :, 1],
            0.25, 0.25, ALU.mult, ALU.add)
        t = sml.tile([128, 4], F32, tag="t", name="t")
        nc.gpsimd.tensor_tensor(
            t[:], u[:], th[:].rearrange("p (i two) -> p i two", two=2)[:, :, 0],
            ALU.mult)
        w2 = sml.tile([128, 8], F32, tag="w2", name="w2")
        nc.gpsimd.tensor_tensor(
            w2[:].rearrange("p (i two) -> p i two", two=2)[:, :, 0],
            u[:], t[:], ALU.add)
        nc.gpsimd.tensor_tensor(
            w2[:].rearrange("p (i two) -> p i two", two=2)[:, :, 1],
            u[:], t[:], ALU.subtract)
        dinv = sml.tile([128, 32], F16, tag="dinv", name="dinv")
        with nc.allow_low_precision(reason="f16 1/den"):
            nc.vector.reciprocal(dinv[:], den[:])
        al = sml.tile([128, 32], F16, tag="al", name="al")
        nc.gpsimd.tensor_tensor(
            al[:].rearrange("p (i t h) -> p i t h", i=4, t=2),
            dinv[:].rearrange("p (i t h) -> p i t h", i=4, t=2),
            w2[:].rearrange("p (i t) -> p i t", i=4)
            .unsqueeze(3).broadcast_to([128, 4, 2, 4]),
            ALU.mult)
        ep = big.tile([128, 2048], F16, tag="ep", name="ep", bufs=3)
        for p in range(2):
            sl = slice(p * 1024, (p + 1) * 1024)
            nc.gpsimd.tensor_tensor(
                ep[:, sl].rearrange("p (i g s) -> p i g s", i=2, g=8),
                e[:, sl].rearrange("p (i g s) -> p i g s", i=2, g=8),
                al[:, p * 16:(p + 1) * 16]
                .rearrange("p (i g) -> p i g", i=2)
                .unsqueeze(3).broadcast_to([128, 2, 8, 64]),
                ALU.mult)
        s["ep"] = ep

    def transp_mm(m, p):
        s = st[m]
        if "etp" not in s:
            s["etp"] = [None, None]
        etp = ps_et.tile([128, 1024], MM_DT, tag="etp", name="etp")
        ep = s["ep"]
        for j in range(2):
            i = 2 * p + j
            for q in range(4):
                nc.tensor.matmul(etp[:, j * 512 + q * 128:j * 512 + (q + 1) * 128],
                                 ep[:, i * 512 + q * 128:i * 512 + (q + 1) * 128],
                                 ident[:], is_transpose=True,
                                 start=(j == 0 and q == 0),
                                 stop=(j == 1 and q == 3))
        s["etp"][p] = etp

    def etcopy(m, p, engine="vector"):
        s = st[m]
        if "eT" not in s:
            s["eT"] = big.tile([128, 2048], MM_DT, tag="eT", name="eT")
        eT = s["eT"]
        dst = eT[:].rearrange("p (q i r) -> p q i r", q=4, i=4)[:, :, 2 * p:2 * p + 2]
        src = s["etp"][p][:].rearrange("p (j q r) -> p q j r", j=2, q=4)
        if engine == "vector":
            nc.vector.tensor_copy(dst, src)
        else:
            nc.scalar.copy(dst, src)

    def fin_mm(m, p):
        s = st[m]
        if "fin" not in s:
            s["fin"] = [None, None]
        eT = s["eT"]
        fin = ps_fin.tile([128, 512], F32, tag="fin", name="fin")
        for j in range(2):
            i = 2 * p + j
            for q in range(4):
                nc.tensor.matmul(fin[:, j * 256:(j + 1) * 256],
                                 eT[:, q * 512 + i * 128:q * 512 + (i + 1) * 128],
                                 v4[q][:], start=(q == 0), stop=(q == 3))
        s["fin"][p] = fin

    def out_step(m, p):
        s = st[m]
        r0 = m * MACRO
        ob = outp.tile([128, 512], F16, tag="ob", name="ob")
        nc.scalar.copy(ob[:], s["fin"][p][:])
        nc.sync.dma_start(
            out_d[r0 + 2 * p * SUB:r0 + (2 * p + 2) * SUB, :]
            .rearrange("(j r) f -> r j f", j=2),
            ob[:].rearrange("p (j f) -> p j f", j=2))

    def ct_dma(m):
        s = st.setdefault(m, {})
        ct = [cin.tile([128, MACRO], MM_DT, tag=f"ct{k}", name=f"ct{k}")
              for k in range(2)]
        for k in range(2):
            nc.sync.dma_start(ct[k][:], cT_d[k * 128:(k + 1) * 128,
                                             m * MACRO:(m + 1) * MACRO])
        s["ct"] = ct

    # ---------------- prologue: norms for macros 0-3 ----------------
    ct_dma(0)
    qp_mm(0)
    norms_sq(0)
    norms_red(0)
    norms_rsqrt(0, 1)
    for m in range(1, min(4, n_macro)):
        qp_mm(m)
        norms_sq(m)
        norms_red(m)
    if n_macro > 1:
        norms_rsqrt(1, min(4, n_macro))

    # ---------------- main loop ----------------
    # FIFO orders are chosen so the PE never waits: v1(0/1) precede the
    # etcopies on vector; gc pair 1 is emitted after qp so tanh-p0 lands
    # before the PE needs the gc psum buffer back.
    for m in range(n_macro):
        if m + 1 < n_macro:
            ct_dma(m + 1)
        sims_mm(m, 0)                    # PE
        sims_mm(m, 1)                    # PE
        v1_mm(m, 0)                      # vector
        v1_mm(m, 1)                      # vector
        if m >= 3:
            out_step(m - 3, 0)           # scalar + DMA
            out_step(m - 3, 1)
        if m >= 2:
            transp_mm(m - 2, 0)          # PE
        gc_mm(m, 0)                      # PE
        if m >= 2:
            etcopy(m - 2, 0, "scalar")   # scalar (vector is the wall)
            transp_mm(m - 2, 1)          # PE
        sims_mm(m, 2)                    # PE
        sims_mm(m, 3)                    # PE
        v1_mm(m, 2)                      # vector
        v1_mm(m, 3)                      # vector
        tanh_p(m, 0)                     # scalar
        exp_p(m, 0)                      # scalar
        if m + 4 < n_macro:
            qp_mm(m + 4)                 # PE
        gc_mm(m, 1)                      # PE
        if m >= 2:
            etcopy(m - 2, 1)             # vector
            fin_mm(m - 2, 0)             # PE
            fin_mm(m - 2, 1)             # PE
        tanh_p(m, 1)                     # scalar
        exp_p(m, 1)                      # scalar
        if 4 <= m + 3 < n_macro:
            norms_red(m + 3)             # vector (fills pre-den bubble)
        if m >= 3 and m % 3 == 0 and m + 1 < n_macro:
            norms_rsqrt_gp(m + 1, min(m + 4, n_macro))
        stats(m)                         # vector + gpsimd
        if m + 4 < n_macro:
            norms_sq(m + 4)              # scalar
        conf_red(m)                      # vector (late: avoids HOL on v1s)
        soft_b(m)
        if m >= 3:
            st.pop(m - 3, None)

    # ---------------- epilogue (interleaved tail) ----------------
    if n_macro >= 3:
        out_step(n_macro - 3, 0)
        out_step(n_macro - 3, 1)
    ma, mb = n_macro - 2, n_macro - 1
    transp_mm(ma, 0)
    etcopy(ma, 0)
    transp_mm(ma, 1)
    etcopy(ma, 1)
    transp_mm(mb, 0)
    etcopy(mb, 0)
    fin_mm(ma, 0)
    fin_mm(ma, 1)
    transp_mm(mb, 1)
    etcopy(mb, 1)
    out_step(ma, 0)
    out_step(ma, 1)
    fin_mm(mb, 0)
    fin_mm(mb, 1)
    out_step(mb, 0)
    out_step(mb, 1)


_CACHE = {}


def _get_program(n_macro=N_MACRO, num_devices=N_CORES):
    key = ("nc", n_macro)
    if key in _CACHE:
        return _CACHE[key]
    rows = n_macro * MACRO
    nc = bacc.Bacc("TRN2", target_bir_lowering=False, debug=False,
                   num_devices=num_devices)
    x8_d = nc.dram_tensor("x8", [128, 2, rows], F8, kind="ExternalInput").ap()
    cT_d = nc.dram_tensor("cT", [D, rows], MM_DT, kind="ExternalInput").ap()
    wqt8_d = nc.dram_tensor("wqt8", [128, 2, 256], F8, kind="ExternalInput").ap()
    mfd8_d = nc.dram_tensor("mfd8", [128, 2, 512], F8, kind="ExternalInput").ap()
    wgc_d = nc.dram_tensor("wgc", [2, 128, 384], MM_DT, kind="ExternalInput").ap()
    v4_d = nc.dram_tensor("v4", [4, 128, 256], MM_DT, kind="ExternalInput").ap()
    wc2r_d = nc.dram_tensor("wc2r", [128, 128], F16, kind="ExternalInput").ap()
    misc_d = nc.dram_tensor("misc", [128, 4], F32, kind="ExternalInput").ap()
    nra_d = nc.dram_tensor("nra", [128, 4], F32, kind="ExternalInput").ap()
    nrb_d = nc.dram_tensor("nrb", [128, 4], F32, kind="ExternalInput").ap()
    ident_d = nc.dram_tensor("identr", [128, 128], MM_DT, kind="ExternalInput").ap()
    out_d = nc.dram_tensor("out", [rows, D], F16, kind="ExternalOutput").ap()
    io = (x8_d, cT_d, wqt8_d, mfd8_d, wgc_d, v4_d, wc2r_d, misc_d,
          nra_d, nrb_d, ident_d, out_d)
    with tile.TileContext(nc) as tc:
        with ExitStack() as ctx:
            _build_kernel(tc, ctx, io, n_macro=n_macro)
    nc.compile()
    _CACHE[key] = nc
    return nc


def _host_consts(fast_keys, fast_vals, deep_keys, deep_vals, Wq, Wg, Wc1, Wc2,
                 Wo, mix_logit, bc2):
    f32 = np.float32

    def norm_keys(k):
        n = np.linalg.norm(k.astype(np.float64), axis=-1, keepdims=True)
        return (k / (n + EPS)).astype(f32)

    knf, knd = norm_keys(fast_keys), norm_keys(deep_keys)
    # M_FD[f, t*256 + h*64 + s] = sum_d Wq[h*64+d, f] * kn_t[h, s, d]
    mfd = np.zeros((D, 512), f32)
    for t, kn in enumerate((knf, knd)):
        for h in range(H):
            wq_h = Wq[h * HD:(h + 1) * HD, :]          # [hd, f]
            mfd[:, t * 256 + h * 64: t * 256 + (h + 1) * 64] = wq_h.T @ kn[h].T

    wgc = np.concatenate([Wg.T, Wc1.T], axis=1)        # [256, 384]
    wgc2 = np.ascontiguousarray(wgc.reshape(2, 128, 384))

    # fp8 DoubleRow operands, x16 pre-scale (cancels via 1/|q|)
    wqt8 = np.ascontiguousarray(
        (W8SCALE * Wq.T).reshape(2, 128, 256).transpose(1, 0, 2)).astype(NP_F8)
    mfd8 = np.ascontiguousarray(
        (W8SCALE * mfd).reshape(2, 128, 512).transpose(1, 0, 2)).astype(NP_F8)

    # wtil[q=(t,c)][(hl*64+s), o] = sum_d vals_t[2c+hl, s, d] * Wo[o, (2c+hl)*64+d]
    v4 = np.zeros((4, 128, 256), f32)
    Wo64 = Wo.astype(np.float64)
    for t, vals in enumerate((fast_vals, deep_vals)):
        for c in range(2):
            for hl in range(2):
                h = 2 * c + hl
                v4[t * 2 + c, hl * 64:(hl + 1) * 64, :] = (
                    vals[h].astype(np.float64) @ Wo64[:, h * 64:(h + 1) * 64].T
                ).astype(f32)
    wc2r = np.ascontiguousarray(np.broadcast_to(Wc2, (128, 128))).astype(f32)
    misc = np.zeros((128, 4), f32)
    misc[:, 0] = f32(mix_logit) / 2
    misc[:, 1] = f32(bc2[0]) / 2
    E = np.array([np.sum((W8SCALE * Wq[h * HD:(h + 1) * HD, :]) ** 2.0)
                  for h in range(H)], np.float64)
    nra = np.ascontiguousarray(
        np.broadcast_to(1.5 / np.sqrt(E), (128, 4))).astype(f32)
    nrb = np.ascontiguousarray(
        np.broadcast_to(0.5 / E ** 1.5, (128, 4))).astype(f32)
    return wqt8, mfd8, wgc2, v4, wc2r, misc, nra, nrb


def kernel(query, context, fast_keys, fast_vals, deep_keys, deep_vals,
           Wq, bq, Wg, bg, Wc1, bc1, Wc2, bc2, Wo, bo, Ws, bs,
           mix_logit, surprise_mean, surprise_var):
    assert not np.any(bq) and not np.any(bg) and not np.any(bc1) \
        and not np.any(bo), "zero-bias fast path only"
    query = np.asarray(query, np.float32)
    context = np.asarray(context, np.float32)

    wqt8, mfd8, wgc2, v4, wc2r, misc, nra, nrb = _host_consts(
        np.asarray(fast_keys, np.float32), np.asarray(fast_vals, np.float32),
        np.asarray(deep_keys, np.float32), np.asarray(deep_vals, np.float32),
        np.asarray(Wq, np.float32), np.asarray(Wg, np.float32),
        np.asarray(Wc1, np.float32), np.asarray(Wc2, np.float32),
        np.asarray(Wo, np.float32), np.asarray(mix_logit, np.float32),
        np.asarray(bc2, np.float32))

    # x8[p, j, r] = query[r, j*128 + p] in fp8
    x8 = np.ascontiguousarray(
        query.T.reshape(2, 128, B).transpose(1, 0, 2)).astype(NP_F8)
    cT = to_mm(context.T)
    wgc2, v4, wc2r_m = (to_mm(a) for a in (wgc2, v4, wc2r))

    identr = to_mm(np.eye(128, dtype=np.float32))
    nc = _get_program()
    in_maps = []
    for c in range(N_CORES):
        sl = slice(c * RPC, (c + 1) * RPC)
        in_maps.append({
            "x8": np.ascontiguousarray(x8[:, :, sl]),
            "cT": np.ascontiguousarray(cT[:, sl]),
            "wqt8": wqt8, "mfd8": mfd8, "wgc": wgc2,
            "v4": v4, "wc2r": wc2r_m, "misc": misc,
            "nra": nra, "nrb": nrb, "identr": identr,
        })
    res = run_bass_kernel_spmd(nc, in_maps, list(range(N_CORES)))
    _CACHE["last_res"] = res
    out = np.concatenate([res.results[c]["out"] for c in range(N_CORES)],
                         axis=0).astype(np.float32)
    return out


# revision 34
# speedup vs baseline: 1.0275x; 1.0275x over previous
"""DualTierMiras Trainium2 kernel (8-core data-parallel), v10 (~184us).

Math (per row r of B=65536, D=256, H=4 heads, hd=64, S=64 keys, 2 banks):
  q = query @ Wq.T
  per head h, bank t: sim = (q_h/|q_h|) . kn_t[h,s,:]   (kn = normalized keys)
  attn = softmax_s(sim);  v_t[h] = attn @ vals_t[h]
  mix  = sigmoid(mix_logit + mean(tanh(context @ Wg.T)))
  conf = sigmoid(Wc2 @ tanh(Wc1 @ context) + bc2)
  out  = (conf*mix*v_fast + conf*(1-mix)*v_deep) @ Wo.T

v3 structure (per core 8192 rows; macro = 512 rows, 4 subtiles):
  - query path (sims + q-norm projections) runs in fp8e4m3 DoubleRow
    matmuls (K=256 folded into one PE pass). wqt/mfd are scaled x16 on the
    host so fp8 stays in normal range; the x16 cancels exactly through the
    1/|q| softmax temperature.
  - context path (gate/conf) and the AV matmuls stay f16 for accuracy.
  - PE stream for step m interleaves transposes/AV of macro m-2 into the
    sims/gc/qp stream of macro m (2-step lag) -> no head-of-line blocking.
  - q-norms: qp/squares run 4 steps ahead; the norm reduce is deferred
    one further step so it fills the vector bubble while scalar runs exp;
    sqrt+recip batched per 3 macros (ACT table set cycles only 8x --
    Sqrt lives in a different table set than Tanh/Exp/Square).
  - all row-stat reduces write f16 outs; output is f16 (host upcasts).
  - e-tile triple-buffered (exp must not wait on the gpsimd ep multiply of
    macro m-2); conf reduction emitted late in the vector FIFO; epilogue
    interleaves the last two macros; critical-path DMAs (wqt8, x8 chunk 0)
    issued first.
"""

import sys

import numpy as np

sys.path.insert(0, "/opt/trn_rl_repo")

from contextlib import ExitStack

import ml_dtypes

import concourse.mybir as mybir
from concourse import bacc, tile
from concourse.bass_utils import run_bass_kernel_spmd

F32 = mybir.dt.float32
F16 = mybir.dt.float16
F8 = mybir.dt.float8e4

N_CORES = 8
B, D, H, S, HD = 65536, 256, 4, 64, 64
RPC = B // N_CORES            # rows per core
MACRO = 512                   # rows per macro tile
SUB = 128                     # rows per sub tile
N_MACRO = RPC // MACRO
N_SUB = MACRO // SUB
EPS = 1e-8
W8SCALE = 16.0                # fp8 weight pre-scale (cancels via 1/|q|)

MM_DT = F16

AF = mybir.ActivationFunctionType
ALU = mybir.AluOpType
DR = mybir.MatmulPerfMode.DoubleRow

NP_F8 = ml_dtypes.float8_e4m3


def to_mm(x):
    return np.ascontiguousarray(x, np.float16)


def _build_kernel(tc, ctx, io, n_macro=N_MACRO):
    nc = tc.nc
    (x8_d, cT_d, wqt8_d, mfd8_d, wgc_d, v4_d, wc2r_d, misc_d,
     ident_d, out_d) = io

    consts = ctx.enter_context(tc.tile_pool(name="consts", bufs=1))
    wqt8 = consts.tile([128, 2, 256], F8, tag="wqt8", name="wqt8")
    mfd8 = consts.tile([128, 2, 512], F8, tag="mfd8", name="mfd8")
    wgc = [consts.tile([128, 384], MM_DT, tag=f"wgc{k}", name=f"wgc{k}") for k in range(2)]
    v4 = [consts.tile([128, 256], MM_DT, tag=f"v4{q}", name=f"v4{q}") for q in range(4)]
    wc2r = consts.tile([128, 128], F16, tag="wc2r", name="wc2r")
    misc = consts.tile([128, 4], F32, tag="misc", name="misc")  # col0 mix/2, col1 bc2/2
    ident = consts.tile([128, 128], MM_DT, tag="ident", name="ident")

    rows = n_macro * MACRO

    # resident x (fp8, DoubleRow layout [128, 2, rows]) in per-1024 chunks
    XCH = 1024
    n_xch = rows // XCH
    xin = ctx.enter_context(tc.tile_pool(name="xin", bufs=1))
    x8c = [xin.tile([128, 2, XCH], F8, tag=f"x8_{c}", name=f"x8_{c}")
           for c in range(n_xch)]
    # critical-path DMAs first: qp(0) needs wqt8 + x8 chunk 0
    nc.sync.dma_start(wqt8[:], wqt8_d[:])
    for c in range(2):
        nc.sync.dma_start(x8c[c][:], x8_d[:, :, c * XCH:(c + 1) * XCH])
    nc.sync.dma_start(mfd8[:], mfd8_d[:])
    for k in range(2):
        nc.sync.dma_start(wgc[k][:], wgc_d[k])
    for q in range(4):
        nc.sync.dma_start(v4[q][:], v4_d[q])
    nc.sync.dma_start(wc2r[:], wc2r_d[:])
    nc.sync.dma_start(misc[:], misc_d[:])
    nc.sync.dma_start(ident[:], ident_d[:])
    for c in range(2, n_xch):
        nc.sync.dma_start(x8c[c][:], x8_d[:, :, c * XCH:(c + 1) * XCH])

    def x8slice(m, lo, hi):
        c = (m * MACRO) // XCH
        o = (m * MACRO) % XCH
        return x8c[c][:, :, o + lo:o + hi]

    cin = ctx.enter_context(tc.tile_pool(name="cin", bufs=4))
    # psum pools: 7 of 8 banks
    ps_a = ctx.enter_context(tc.tile_pool(name="ps_a", bufs=2, space="PSUM"))
    ps_gc = ctx.enter_context(tc.tile_pool(name="ps_gc", bufs=1, space="PSUM"))
    ps_et = ctx.enter_context(tc.tile_pool(name="ps_et", bufs=1, space="PSUM"))
    ps_fin = ctx.enter_context(tc.tile_pool(name="ps_fin", bufs=2, space="PSUM"))

    big = ctx.enter_context(tc.tile_pool(name="big", bufs=2))
    sml = ctx.enter_context(tc.tile_pool(name="sml", bufs=3))
    outp = ctx.enter_context(tc.tile_pool(name="outp", bufs=4))
    stat = ctx.enter_context(tc.tile_pool(name="stat", bufs=1))

    # resident norm stats for the whole core
    ssq_all = stat.tile([128, 16 * n_macro], F32, tag="ssq_all", name="ssq_all")
    invna_all = stat.tile([128, 16 * n_macro], F32, tag="invna_all",
                          name="invna_all")

    st = {}

    def sims_mm(m, i):
        s = st.setdefault(m, {})
        if "sim" not in s:
            s["sim"] = [None] * N_SUB
        sim = ps_a.tile([128, 512], F32, tag="blk", name="sim")
        nc.tensor.matmul(sim[:], x8slice(m, i * SUB, (i + 1) * SUB), mfd8[:],
                         start=True, stop=True, perf_mode=DR)
        s["sim"][i] = sim

    def gc_mm(m, p):
        s = st[m]
        if "gc" not in s:
            s["gc"] = [None, None]
        ct = s["ct"]
        gcp = ps_gc.tile([128, 1024], F32, tag="gcp", name="gcp")
        for j in range(2):
            i = 2 * p + j
            sl = slice(i * SUB, (i + 1) * SUB)
            o = j * 512
            nc.tensor.matmul(gcp[:, o:o + 384], ct[0][:, sl], wgc[0][:],
                             start=True, stop=False)
            nc.tensor.matmul(gcp[:, o:o + 384], ct[1][:, sl], wgc[1][:],
                             start=False, stop=True)
        s["gc"][p] = gcp

    def qp_mm(m):
        """fp8-DR q projections for macro m (emitted 4 steps ahead)."""
        s = st.setdefault(m, {})
        s["qp"] = []
        for p in range(2):
            qpr = ps_a.tile([128, 512], F32, tag="blk", name="qpr")
            for j in range(2):
                i = 2 * p + j
                nc.tensor.matmul(qpr[:, j * 256:(j + 1) * 256],
                                 x8slice(m, i * SUB, (i + 1) * SUB), wqt8[:],
                                 start=True, stop=True, perf_mode=DR)
            s["qp"].append(qpr)

    def norms_sq(m):
        """squares (scalar) for macro m's q projections."""
        s = st[m]
        qsq = big.tile([128, 1024], F16, tag="qsq", name="qsq")
        for p in range(2):
            nc.scalar.activation(qsq[:, p * 512:(p + 1) * 512], s["qp"][p][:],
                                 AF.Square)
        s["qsq"] = qsq

    def norms_red(m):
        """per-head reduce into ssq_all (deferred one step: fills the
        vector bubble while scalar runs the current macro's exp)."""
        nc.vector.reduce_sum(
            ssq_all[:, m * 16:(m + 1) * 16],
            st[m]["qsq"][:].rearrange("p (g s) -> p g s", g=16),
            axis=mybir.AxisListType.X)

    def norms_rsqrt(m0, m1):
        """batched sqrt+recip for macros [m0, m1) -> invna_all."""
        sl = slice(m0 * 16, m1 * 16)
        sna = sml.tile([128, 16 * 4], F32, tag="sna", name="sna",
                       padded_shape=[128, 64])
        w = (m1 - m0) * 16
        nc.scalar.activation(sna[:, 0:w], ssq_all[:, sl], AF.Sqrt)
        nc.vector.reciprocal(invna_all[:, sl], sna[:, 0:w])

    def v1_mm(m, i):
        """vector: earg slice = sim * invn (per subtile)."""
        s = st[m]
        if "earg" not in s:
            s["earg"] = big.tile([128, 2048], F16, tag="earg", name="earg")
        nc.vector.tensor_tensor(
            s["earg"][:, i * 512:(i + 1) * 512]
            .rearrange("p (t h s) -> p t h s", t=2, h=4),
            s["sim"][i][:].rearrange("p (t h s) -> p t h s", t=2, h=4),
            invna_all[:, m * 16 + i * 4:m * 16 + (i + 1) * 4]
            .unsqueeze(1).unsqueeze(3).broadcast_to([128, 2, 4, 64]),
            ALU.mult)

    def tanh_p(m, p):
        s = st[m]
        if "tg" not in s:
            s["tg"] = big.tile([128, 1536], F16, tag="tg", name="tg")
        nc.scalar.activation(
            s["tg"][:, p * 768:(p + 1) * 768].rearrange("p (j f) -> p j f", j=2),
            s["gc"][p][:].rearrange("p (j f) -> p j f", j=2)[:, :, 0:384],
            AF.Tanh)

    def exp_p(m, p):
        s = st[m]
        if "e" not in s:
            s["e"] = big.tile([128, 2048], F16, tag="e", name="e", bufs=3)
        nc.scalar.activation(s["e"][:, p * 1024:(p + 1) * 1024],
                             s["earg"][:, p * 1024:(p + 1) * 1024], AF.Exp)

    def stats(m):
        """vector den/gate/conf reductions (+gpsimd conf product)."""
        s = st[m]
        e, tg = s["e"], s["tg"]
        den = sml.tile([128, 32], F16, tag="den", name="den")
        thin = sml.tile([128, 8], F16, tag="thin", name="thin")
        with nc.allow_low_precision(reason="f16 softmax stats (<=64 adds)"):
            nc.vector.reduce_sum(
                den[:], e[:].rearrange("p (g s) -> p g s", g=32),
                axis=mybir.AxisListType.X)
            nc.vector.reduce_sum(
                thin[:].rearrange("p (i two) -> p i two", two=2)[:, :, 0:1],
                tg[:].rearrange("p (i f) -> p i f", i=4)[:, :, 0:256]
                .rearrange("p i (one f) -> p i one f", one=1),
                axis=mybir.AxisListType.X)
        cp = big.tile([128, 512], F16, tag="cp", name="cp")
        nc.gpsimd.tensor_tensor(
            cp[:].rearrange("p (i f) -> p i f", i=4),
            tg[:].rearrange("p (i f) -> p i f", i=4)[:, :, 256:384],
            wc2r[:].unsqueeze(1).broadcast_to([128, 4, 128]),
            ALU.mult)
        s["cp"] = cp
        s["den"] = den
        s["thin"] = thin

    def conf_red(m):
        """conf reduction, emitted late so a slow gpsimd cp can't
        head-of-line-block the next step's v1s on the vector queue."""
        s = st[m]
        with nc.allow_low_precision(reason="f16 conf stat"):
            nc.vector.reduce_sum(
                s["thin"][:].rearrange("p (i two) -> p i two", two=2)[:, :, 1:2],
                s["cp"][:].rearrange("p (i one f) -> p i one f", i=4, one=1),
                axis=mybir.AxisListType.X)

    def soft_b(m):
        s = st[m]
        thin, den, e = s["thin"], s["den"], s["e"]
        th = sml.tile([128, 8], F32, tag="th", name="th")
        nc.scalar.activation(
            th[:].rearrange("p (i two) -> p i two", two=2)[:, :, 0:1],
            thin[:].rearrange("p (i two) -> p i two", two=2)[:, :, 0:1],
            AF.Tanh, bias=misc[:, 0:1], scale=1.0 / 512.0)
        nc.scalar.activation(
            th[:].rearrange("p (i two) -> p i two", two=2)[:, :, 1:2],
            thin[:].rearrange("p (i two) -> p i two", two=2)[:, :, 1:2],
            AF.Tanh, bias=misc[:, 1:2], scale=0.5)
        u = sml.tile([128, 4], F32, tag="u", name="u")
        nc.gpsimd.tensor_scalar(
            u[:], th[:].rearrange("p (i two) -> p i two", two=2)[:, :, 1],
            0.25, 0.25, ALU.mult, ALU.add)
        t = sml.tile([128, 4], F32, tag="t", name="t")
        nc.gpsimd.tensor_tensor(
            t[:], u[:], th[:].rearrange("p (i two) -> p i two", two=2)[:, :, 0],
            ALU.mult)
        w2 = sml.tile([128, 8], F32, tag="w2", name="w2")
        nc.gpsimd.tensor_tensor(
            w2[:].rearrange("p (i two) -> p i two", two=2)[:, :, 0],
            u[:], t[:], ALU.add)
        nc.gpsimd.tensor_tensor(
            w2[:].rearrange("p (i two) -> p i two", two=2)[:, :, 1],
            u[:], t[:], ALU.subtract)
        dinv = sml.tile([128, 32], F16, tag="dinv", name="dinv")
        with nc.allow_low_precision(reason="f16 1/den"):
            nc.vector.reciprocal(dinv[:], den[:])
        al = sml.tile([128, 32], F16, tag="al", name="al")
        nc.gpsimd.tensor_tensor(
            al[:].rearrange("p (i t h) -> p i t h", i=4, t=2),
            dinv[:].rearrange("p (i t h) -> p i t h", i=4, t=2),
            w2[:].rearrange("p (i t) -> p i t", i=4)
            .unsqueeze(3).broadcast_to([128, 4, 2, 4]),
            ALU.mult)
        ep = big.tile([128, 2048], F16, tag="ep", name="ep", bufs=3)
        for p in range(2):
            sl = slice(p * 1024, (p + 1) * 1024)
            nc.gpsimd.tensor_tensor(
                ep[:, sl].rearrange("p (i g s) -> p i g s", i=2, g=8),
                e[:, sl].rearrange("p (i g s) -> p i g s", i=2, g=8),
                al[:, p * 16:(p + 1) * 16]
                .rearrange("p (i g) -> p i g", i=2)
                .unsqueeze(3).broadcast_to([128, 2, 8, 64]),
                ALU.mult)
        s["ep"] = ep

    def transp_mm(m, p):
        s = st[m]
        if "etp" not in s:
            s["etp"] = [None, None]
        etp = ps_et.tile([128, 1024], MM_DT, tag="etp", name="etp")
        ep = s["ep"]
        for j in range(2):
            i = 2 * p + j
            for q in range(4):
                nc.tensor.matmul(etp[:, j * 512 + q * 128:j * 512 + (q + 1) * 128],
                                 ep[:, i * 512 + q * 128:i * 512 + (q + 1) * 128],
                                 ident[:], is_transpose=True,
                                 start=(j == 0 and q == 0),
                                 stop=(j == 1 and q == 3))
        s["etp"][p] = etp

    def etcopy(m, p, engine="vector"):
        s = st[m]
        if "eT" not in s:
            s["eT"] = big.tile([128, 2048], MM_DT, tag="eT", name="eT")
        eT = s["eT"]
        dst = eT[:].rearrange("p (q i r) -> p q i r", q=4, i=4)[:, :, 2 * p:2 * p + 2]
        src = s["etp"][p][:].rearrange("p (j q r) -> p q j r", j=2, q=4)
        if engine == "vector":
            nc.vector.tensor_copy(dst, src)
        else:
            nc.scalar.copy(dst, src)

    def fin_mm(m, p):
        s = st[m]
        if "fin" not in s:
            s["fin"] = [None, None]
        eT = s["eT"]
        fin = ps_fin.tile([128, 512], F32, tag="fin", name="fin")
        for j in range(2):
            i = 2 * p + j
            for q in range(4):
                nc.tensor.matmul(fin[:, j * 256:(j + 1) * 256],
                                 eT[:, q * 512 + i * 128:q * 512 + (i + 1) * 128],
                                 v4[q][:], start=(q == 0), stop=(q == 3))
        s["fin"][p] = fin

    def out_step(m, p):
        s = st[m]
        r0 = m * MACRO
        ob = outp.tile([128, 512], F16, tag="ob", name="ob")
        nc.scalar.copy(ob[:], s["fin"][p][:])
        nc.sync.dma_start(
            out_d[r0 + 2 * p * SUB:r0 + (2 * p + 2) * SUB, :]
            .rearrange("(j r) f -> r j f", j=2),
            ob[:].rearrange("p (j f) -> p j f", j=2))

    def ct_dma(m):
        s = st.setdefault(m, {})
        ct = [cin.tile([128, MACRO], MM_DT, tag=f"ct{k}", name=f"ct{k}")
              for k in range(2)]
        for k in range(2):
            nc.sync.dma_start(ct[k][:], cT_d[k * 128:(k + 1) * 128,
                                             m * MACRO:(m + 1) * MACRO])
        s["ct"] = ct

    # ---------------- prologue: norms for macros 0-3 ----------------
    ct_dma(0)
    qp_mm(0)
    norms_sq(0)
    norms_red(0)
    norms_rsqrt(0, 1)
    for m in range(1, min(4, n_macro)):
        qp_mm(m)
        norms_sq(m)
        norms_red(m)
    if n_macro > 1:
        norms_rsqrt(1, min(4, n_macro))

    # ---------------- main loop ----------------
    # FIFO orders are chosen so the PE never waits: v1(0/1) precede the
    # etcopies on vector; gc pair 1 is emitted after qp so tanh-p0 lands
    # before the PE needs the gc psum buffer back.
    for m in range(n_macro):
        if m + 1 < n_macro:
            ct_dma(m + 1)
        sims_mm(m, 0)                    # PE
        sims_mm(m, 1)                    # PE
        v1_mm(m, 0)                      # vector
        v1_mm(m, 1)                      # vector
        if m >= 3:
            out_step(m - 3, 0)           # scalar + DMA
            out_step(m - 3, 1)
        if m >= 2:
            transp_mm(m - 2, 0)          # PE
        gc_mm(m, 0)                      # PE
        if m >= 2:
            etcopy(m - 2, 0, "scalar")   # scalar (vector is the wall)
            transp_mm(m - 2, 1)          # PE
        sims_mm(m, 2)                    # PE
        sims_mm(m, 3)                    # PE
        v1_mm(m, 2)                      # vector
        v1_mm(m, 3)                      # vector
        tanh_p(m, 0)                     # scalar
        exp_p(m, 0)                      # scalar
        if m + 4 < n_macro:
            qp_mm(m + 4)                 # PE
        gc_mm(m, 1)                      # PE
        if m >= 2:
            etcopy(m - 2, 1)             # vector
            fin_mm(m - 2, 0)             # PE
            fin_mm(m - 2, 1)             # PE
        tanh_p(m, 1)                     # scalar
        exp_p(m, 1)                      # scalar
        if 4 <= m + 3 < n_macro:
            norms_red(m + 3)             # vector (fills pre-den bubble)
        if m >= 3 and m % 3 == 0 and m + 1 < n_macro:
            norms_rsqrt(m + 1, min(m + 4, n_macro))
        stats(m)                         # vector + gpsimd
        if m + 4 < n_macro:
            norms_sq(m + 4)              # scalar
        conf_red(m)                      # vector (late: avoids HOL on v1s)
        soft_b(m)
        if m >= 3:
            st.pop(m - 3, None)

    # ---------------- epilogue (interleaved tail) ----------------
    if n_macro >= 3:
        out_step(n_macro - 3, 0)
        out_step(n_macro - 3, 1)
    ma, mb = n_macro - 2, n_macro - 1
    transp_mm(ma, 0)
    etcopy(ma, 0)
    transp_mm(ma, 1)
    etcopy(ma, 1)
    transp_mm(mb, 0)
    etcopy(mb, 0)
    fin_mm(ma, 0)
    fin_mm(ma, 1)
    transp_mm(mb, 1)
    etcopy(mb, 1)
    out_step(ma, 0)
    out_step(ma, 1)
    fin_mm(mb, 0)
    fin_mm(mb, 1)
    out_step(mb, 0)
    out_step(mb, 1)


_CACHE = {}


def _get_program(n_macro=N_MACRO, num_devices=N_CORES):
    key = ("nc", n_macro)
    if key in _CACHE:
        return _CACHE[key]
    rows = n_macro * MACRO
    nc = bacc.Bacc("TRN2", target_bir_lowering=False, debug=False,
                   num_devices=num_devices)
    x8_d = nc.dram_tensor("x8", [128, 2, rows], F8, kind="ExternalInput").ap()
    cT_d = nc.dram_tensor("cT", [D, rows], MM_DT, kind="ExternalInput").ap()
    wqt8_d = nc.dram_tensor("wqt8", [128, 2, 256], F8, kind="ExternalInput").ap()
    mfd8_d = nc.dram_tensor("mfd8", [128, 2, 512], F8, kind="ExternalInput").ap()
    wgc_d = nc.dram_tensor("wgc", [2, 128, 384], MM_DT, kind="ExternalInput").ap()
    v4_d = nc.dram_tensor("v4", [4, 128, 256], MM_DT, kind="ExternalInput").ap()
    wc2r_d = nc.dram_tensor("wc2r", [128, 128], F16, kind="ExternalInput").ap()
    misc_d = nc.dram_tensor("misc", [128, 4], F32, kind="ExternalInput").ap()
    ident_d = nc.dram_tensor("identr", [128, 128], MM_DT, kind="ExternalInput").ap()
    out_d = nc.dram_tensor("out", [rows, D], F16, kind="ExternalOutput").ap()
    io = (x8_d, cT_d, wqt8_d, mfd8_d, wgc_d, v4_d, wc2r_d, misc_d,
          ident_d, out_d)
    with tile.TileContext(nc) as tc:
        with ExitStack() as ctx:
            _build_kernel(tc, ctx, io, n_macro=n_macro)
    nc.compile()
    _CACHE[key] = nc
    return nc


def _host_consts(fast_keys, fast_vals, deep_keys, deep_vals, Wq, Wg, Wc1, Wc2,
                 Wo, mix_logit, bc2):
    f32 = np.float32

    def norm_keys(k):
        n = np.linalg.norm(k.astype(np.float64), axis=-1, keepdims=True)
        return (k / (n + EPS)).astype(f32)

    knf, knd = norm_keys(fast_keys), norm_keys(deep_keys)
    # M_FD[f, t*256 + h*64 + s] = sum_d Wq[h*64+d, f] * kn_t[h, s, d]
    mfd = np.zeros((D, 512), f32)
    for t, kn in enumerate((knf, knd)):
        for h in range(H):
            wq_h = Wq[h * HD:(h + 1) * HD, :]          # [hd, f]
            mfd[:, t * 256 + h * 64: t * 256 + (h + 1) * 64] = wq_h.T @ kn[h].T

    wgc = np.concatenate([Wg.T, Wc1.T], axis=1)        # [256, 384]
    wgc2 = np.ascontiguousarray(wgc.reshape(2, 128, 384))

    # fp8 DoubleRow operands, x16 pre-scale (cancels via 1/|q|)
    wqt8 = np.ascontiguousarray(
        (W8SCALE * Wq.T).reshape(2, 128, 256).transpose(1, 0, 2)).astype(NP_F8)
    mfd8 = np.ascontiguousarray(
        (W8SCALE * mfd).reshape(2, 128, 512).transpose(1, 0, 2)).astype(NP_F8)

    # wtil[q=(t,c)][(hl*64+s), o] = sum_d vals_t[2c+hl, s, d] * Wo[o, (2c+hl)*64+d]
    v4 = np.zeros((4, 128, 256), f32)
    Wo64 = Wo.astype(np.float64)
    for t, vals in enumerate((fast_vals, deep_vals)):
        for c in range(2):
            for hl in range(2):
                h = 2 * c + hl
                v4[t * 2 + c, hl * 64:(hl + 1) * 64, :] = (
                    vals[h].astype(np.float64) @ Wo64[:, h * 64:(h + 1) * 64].T
                ).astype(f32)
    wc2r = np.ascontiguousarray(np.broadcast_to(Wc2, (128, 128))).astype(f32)
    misc = np.zeros((128, 4), f32)
    misc[:, 0] = f32(mix_logit) / 2
    misc[:, 1] = f32(bc2[0]) / 2
    return wqt8, mfd8, wgc2, v4, wc2r, misc


def kernel(query, context, fast_keys, fast_vals, deep_keys, deep_vals,
           Wq, bq, Wg, bg, Wc1, bc1, Wc2, bc2, Wo, bo, Ws, bs,
           mix_logit, surprise_mean, surprise_var):
    assert not np.any(bq) and not np.any(bg) and not np.any(bc1) \
        and not np.any(bo), "zero-bias fast path only"
    query = np.asarray(query, np.float32)
    context = np.asarray(context, np.float32)

    wqt8, mfd8, wgc2, v4, wc2r, misc = _host_consts(
        np.asarray(fast_keys, np.float32), np.asarray(fast_vals, np.float32),
        np.asarray(deep_keys, np.float32), np.asarray(deep_vals, np.float32),
        np.asarray(Wq, np.float32), np.asarray(Wg, np.float32),
        np.asarray(Wc1, np.float32), np.asarray(Wc2, np.float32),
        np.asarray(Wo, np.float32), np.asarray(mix_logit, np.float32),
        np.asarray(bc2, np.float32))

    # x8[p, j, r] = query[r, j*128 + p] in fp8
    x8 = np.ascontiguousarray(
        query.T.reshape(2, 128, B).transpose(1, 0, 2)).astype(NP_F8)
    cT = to_mm(context.T)
    wgc2, v4, wc2r_m = (to_mm(a) for a in (wgc2, v4, wc2r))

    identr = to_mm(np.eye(128, dtype=np.float32))
    nc = _get_program()
    in_maps = []
    for c in range(N_CORES):
        sl = slice(c * RPC, (c + 1) * RPC)
        in_maps.append({
            "x8": np.ascontiguousarray(x8[:, :, sl]),
            "cT": np.ascontiguousarray(cT[:, sl]),
            "wqt8": wqt8, "mfd8": mfd8, "wgc": wgc2,
            "v4": v4, "wc2r": wc2r_m, "misc": misc,
            "identr": identr,
        })
    res = run_bass_kernel_spmd(nc, in_maps, list(range(N_CORES)))
    _CACHE["last_res"] = res
    out = np.concatenate([res.results[c]["out"] for c in range(N_CORES)],
                         axis=0).astype(np.float32)
    return out
